# revision 91
# baseline (speedup 1.0000x reference)
"""Trainium2 Bass kernel: causal transformer encoder layer (pre-QKV fused),
SPMD across 8 NeuronCores.

Sharding: core c handles batch b = c//2.  The two cores of a batch split the
2048 query positions into 4 chunks of 256 each, paired so both halves get the
same total causal work AND the same static program structure (SPMD): chunk
slots have structure extents E = [16, 12, 8, 4] k-tiles (of 128); the two
halves' actual extents differ by exactly 2 at each slot, the difference is
absorbed by per-core mask DATA (multiplicative 0/1 masks on exp(scores)).

fp8 (TRN e4m3) + DoubleRow perf mode for every matmul whose contraction is
>= 256 (QKV proj, AV, out-proj, FFN1, FFN2): 2 reduction k-tiles are consumed
per pass, halving PE time.  Scores stay bf16 (contraction is only hd=64, no
DoubleRow gain).  Weights are pre-scaled x64 on the host so fp8 normals are
used; descales are folded into downstream frees (exp scale, relu scale,
scalar_tensor_tensor residual fusions, softmax denominator column value).

Scale bookkeeping:
  wqkv' = 64*wqkv (q part also has 1/sqrt(hd) folded)  -> sc = 4096*scores,
    exp uses scale=1/4096.
  V path: Vaug = [V/16*64=4V | 0.125]; cps = [4*num | denom/8];
    inv = 8/denom;  ctx_stored = 32*ctx  (fp8, ~unit scale).
  wo' = 64*wo -> out-proj PSUM = 2048*attn_out; residual fused as
    tt = ps*(1/2048) + xown  (scalar_tensor_tensor).
  w1' = 64*w1; relu copy scale 1/16 -> hb = 4*relu(x@W1) (fp8).
  w2' = 64*w2 -> FFN2 PSUM = 256*ff; residual tt = yp*(1/256) + xln1.
"""

from contextlib import ExitStack
from dataclasses import dataclass

import numpy as np
import ml_dtypes

import concourse.bass as bass
import concourse.bacc as bacc
import concourse.tile as tile
from concourse import mybir
from concourse.bass_utils import run_bass_kernel_spmd
from concourse.masks import make_identity

F32 = mybir.dt.float32
I32 = mybir.dt.int32
BF16 = mybir.dt.bfloat16
F8 = mybir.dt.float8e4
AF = mybir.ActivationFunctionType
ALU = mybir.AluOpType
DR = mybir.MatmulPerfMode.DoubleRow

EPS = 1e-5
WSCALE = 64.0          # host premultiplier on all weight matrices
VSCALE = 4.0           # net scale of V stored in fp8 (Vaug = 4*V)
DENOM_COL = 0.125      # ones-column value => ctx_stored = (VSCALE/DENOM_COL)*ctx
CTX_SCALE = VSCALE / DENOM_COL  # 32
HB_SCALE = 4.0         # hb = 4*relu(.)


@dataclass
class Cfg:
    B: int = 4
    S: int = 2048
    D: int = 512
    F: int = 2048
    H: int = 8
    CHUNK: int = 256
    KT: int = 128
    # which ops to emit (skip ops that are no-ops for the actual input values)
    use_bq: bool = False
    use_bk: bool = False
    use_bv: bool = False
    use_b1: bool = False
    use_b2: bool = False
    use_g1: bool = False
    use_bn1: bool = False
    use_g2: bool = False
    use_bn2: bool = False

    @property
    def HD(self):
        return self.D // self.H

    @property
    def DK(self):
        return self.D // 128  # number of 128-row tiles of D

    @property
    def FK(self):
        return self.F // 128

    @property
    def NCH(self):
        return self.S // self.CHUNK  # chunks per batch-sequence

    @property
    def NQ(self):
        return (self.NCH // 2) * self.CHUNK  # local query tokens per core

    @property
    def NSLOT(self):
        return self.NCH // 2

    @property
    def QT(self):
        return self.NQ // 128  # local q 128-tiles

    def ext(self, ci):
        return ((ci + 1) * self.CHUNK) // self.KT

    def slot_chunks(self, half):
        n = self.NCH
        if half == 0:
            s = [i for i in range(n) if i % 4 in (0, 3)]
        else:
            s = [i for i in range(n) if i % 4 in (1, 2)]
        return sorted(s, key=lambda ci: -self.ext(ci))

    def slot_qs(self, half):
        return [ci * self.CHUNK for ci in self.slot_chunks(half)]

    def slot_E(self):
        a = self.slot_chunks(0)
        b = self.slot_chunks(1)
        E = [max(self.ext(x), self.ext(y)) for x, y in zip(a, b)]
        for e in E:
            assert e % 4 == 0, E
        return E

    def slot_groups(self, s):
        """k-tile groups (j0, width) for slot s: widths <=6, even, covering
        E[s] tiles.  Chosen so exp instructions are as wide as PSUM allows
        (3 banks = 6*CHUNK fp32 columns)."""
        E = self.slot_E()[s]
        groups = []
        j = 0
        while j < E:
            w = min(6, E - j)
            if w % 2:  # keep widths even for the AV DoubleRow pairing
                w -= 1
            groups.append((j, w))
            j += w
        return groups


def build_nc(cfg: Cfg, n_bodies: int = 1) -> bass.Bass:
    S, D, F, H, HD = cfg.S, cfg.D, cfg.F, cfg.H, cfg.HD
    DK, FK, QT, NQ, CHUNK, KT = cfg.DK, cfg.FK, cfg.QT, cfg.NQ, cfg.CHUNK, cfg.KT
    NSLOT = cfg.NSLOT
    E = cfg.slot_E()
    HPT = 128 // HD  # heads per 128-row tile (2)
    WQ = min(512, NQ)   # moving width for q-token chunks
    WS = min(512, S)    # moving width for full-seq token chunks
    NSUB = WQ // 128

    VW = 80  # per-head column block in Vaug: 64 V + 1 denom + 15 pad
    # (dual-fp8 LDWEIGHTS requires 16-byte-aligned offsets/steps; 65 is not)

    nc = bacc.Bacc("TRN2", target_bir_lowering=False)

    xT_d = nc.declare_dram_parameter("xT", [D, S], F8, isOutput=False)
    xqT_d = nc.declare_dram_parameter("xqT", [D, NQ], F8, isOutput=False)
    xown_d = nc.declare_dram_parameter("xown", [NQ, D], F32, isOutput=False)
    wqkv_d = nc.declare_dram_parameter("wqkvT", [D, 3 * D], F8, isOutput=False)
    wo_d = nc.declare_dram_parameter("woT", [D, D], F8, isOutput=False)
    w1_d = nc.declare_dram_parameter("w1T", [D, F], F8, isOutput=False)
    w2_d = nc.declare_dram_parameter("w2T", [F, D], F8, isOutput=False)
    bq_d = nc.declare_dram_parameter("bq", [D], F32, isOutput=False)
    bk_d = nc.declare_dram_parameter("bk", [D], F32, isOutput=False)
    bv_d = nc.declare_dram_parameter("bv", [D], F32, isOutput=False)
    b1_d = nc.declare_dram_parameter("b1", [F], F32, isOutput=False)
    b2_d = nc.declare_dram_parameter("b2", [128, D], F32, isOutput=False)
    g1_d = nc.declare_dram_parameter("g1v", [128, D], F32, isOutput=False)
    bn1_d = nc.declare_dram_parameter("bn1v", [128, D], F32, isOutput=False)
    g2_d = nc.declare_dram_parameter("g2v", [128, D], F32, isOutput=False)
    bn2_d = nc.declare_dram_parameter("bn2v", [128, D], F32, isOutput=False)
    masks_d = nc.declare_dram_parameter(
        "masks", [128, NSLOT, 4 * CHUNK], F8, isOutput=False
    )
    out_d = nc.declare_dram_parameter("out", [NQ, D], F32, isOutput=True)

    with ExitStack() as top:
        tc = top.enter_context(tile.TileContext(nc, pool_alloc_mode="queue"))
        consts = top.enter_context(tc.tile_pool(name="consts", bufs=1))

        ident = consts.tile([128, 128], BF16)
        make_identity(nc, ident)

        masks_sb = consts.tile([128, NSLOT, 4 * CHUNK], F8)

        eps_sb = consts.tile([128, 1], F32)
        nc.vector.memset(eps_sb, EPS)
        # dummy exp at t=0: pulls the ~1.4us exp-table load off the critical
        # path of the first real softmax exp
        warm_sb = consts.tile([128, 1], F32)
        nc.scalar.activation(out=warm_sb, in_=eps_sb, func=AF.Exp)

        bq_sb = bk_sb = bv_sb = b1_sb = None
        if cfg.use_bq:
            bq_sb = consts.tile([128, DK], F32)
            nc.sync.dma_start(out=bq_sb, in_=bq_d.rearrange("(m p) -> p m", p=128))
        if cfg.use_bk:
            bk_sb = consts.tile([128, DK], F32)
            nc.sync.dma_start(out=bk_sb, in_=bk_d.rearrange("(m p) -> p m", p=128))
        if cfg.use_bv:
            bv_sb = consts.tile([128, DK], F32)
            nc.sync.dma_start(out=bv_sb, in_=bv_d.rearrange("(m p) -> p m", p=128))
        if cfg.use_b1:
            b1_sb = consts.tile([128, FK], F32)
            nc.sync.dma_start(out=b1_sb, in_=b1_d.rearrange("(f p) -> p f", p=128))

        def bcast_const(dram, nm):
            t = consts.tile([128, D], F32, name=nm, tag=nm)
            nc.sync.dma_start(out=t, in_=dram[:, :])
            return t

        g1_b = bcast_const(g1_d, "g1b") if cfg.use_g1 else None
        bn1_b = bcast_const(bn1_d, "bn1b") if cfg.use_bn1 else None
        g2_b = bcast_const(g2_d, "g2b") if cfg.use_g2 else None
        bn2_b = bcast_const(bn2_d, "bn2b") if cfg.use_bn2 else None
        b2_b = bcast_const(b2_d, "b2b") if cfg.use_b2 else None

        def emit_body():
            # long-lived tiles live in pctx (opened first, closed last)
            pctx_cm = tc.tile_pool(name="pctx", bufs=1)
            pctx = pctx_cm.__enter__()
            ctx_fm = pctx.tile([128, DK, NQ], F8)
            wo_sb = pctx.tile([128, DK, D], F8)
            xown_sb = pctx.tile([128, QT, D], F32)
            w1_sb = pctx.tile([128, DK, F], F8)
            w2_sb = pctx.tile([128, FK, D], F8)
            xln1 = pctx.tile([128, QT, D], BF16)
            x1tb = pctx.tile([128, DK, NQ], BF16)
            x1t = pctx.tile([128, DK, NQ], F8)
            tt_all = pctx.tile([128, QT, D], F32)
            mv_all = pctx.tile([128, QT, 2], F32)
            rstd_all = pctx.tile([128, QT], F32)


            pqkv_cm = tc.tile_pool(name="pqkv", bufs=1)
            pqkv = pqkv_cm.__enter__()
            Qfm = pqkv.tile([128, DK, NQ], BF16)
            Kfm = pqkv.tile([128, DK, S], BF16)
            Vaug = pqkv.tile([128, S // KT, H * VW], F8)

            # ---------------- Phase A: QKV projections --------------------------
            pa_cm = tc.tile_pool(name="pa", bufs=1)
            pa = pa_cm.__enter__()

            xT_sb = pa.tile([128, DK, S], F8)
            xqT_sb = pa.tile([128, DK, NQ], F8)
            wqkv_sb = pa.tile([128, DK, 3 * D], F8)
            xT_r = xT_d.rearrange("(m p) t -> p m t", p=128)
            xqT_r = xqT_d.rearrange("(m p) t -> p m t", p=128)
            wqkv_r = wqkv_d.rearrange("(m p) c -> p m c", p=128)
            for k in range(2):
                nc.sync.dma_start(out=xT_sb[:, k, :], in_=xT_r[:, k, :])
                nc.scalar.dma_start(out=xT_sb[:, k + 2, :], in_=xT_r[:, k + 2, :])
                nc.gpsimd.dma_start(out=wqkv_sb[:, k, :], in_=wqkv_r[:, k, :])
            for k in range(2, DK):
                nc.gpsimd.dma_start(out=wqkv_sb[:, k, :], in_=wqkv_r[:, k, :])
            for k in range(DK):
                nc.sync.dma_start(out=xqT_sb[:, k, :], in_=xqT_r[:, k, :])
            nc.sync.dma_start(out=masks_sb, in_=masks_d[:, :, :])
            # late weights follow on the gpsimd queue (needed from mid-phase-B)
            nc.gpsimd.dma_start(out=wo_sb, in_=wo_d.rearrange("(m p) c -> p m c", p=128))
            nc.gpsimd.dma_start(out=xown_sb, in_=xown_d.rearrange("(t p) d -> p t d", p=128))
            nc.gpsimd.dma_start(out=w1_sb, in_=w1_d.rearrange("(m p) c -> p m c", p=128))
            nc.gpsimd.dma_start(out=w2_sb, in_=w2_d.rearrange("(f p) c -> p f c", p=128))

            # PSUM budget (8 banks): pps 2x1 | pbs 2x2 | pbc 2x1 = 8.
            # pps is the shared 1-bank scratch pool: phase-A QKV accumulators,
            # in-B V projections, out-proj accumulators, LN1 transposes.  After
            # phase B, pbs+pbc close and pfh(4)+pfy(2) open alongside pps.
            pps_cm = tc.tile_pool(name="pps", bufs=2, space="PSUM")
            pps = pps_cm.__enter__()
            pbs_cm = tc.tile_pool(name="pb_sc", bufs=2, space="PSUM")
            pbs = pbs_cm.__enter__()
            pbc_cm = tc.tile_pool(name="pb_cx", bufs=2, space="PSUM")
            pbc = pbc_cm.__enter__()

            def emit_k(m, on_act, split=False, chs=None):
                for ch in (range(S // WS) if chs is None else chs):
                    ch_act = on_act if not split else (ch < 2)
                    ps = pps.tile([128, WS], F32, tag="ps")
                    for k in range(0, DK, 2):
                        nc.tensor.matmul(
                            out=ps,
                            lhsT=wqkv_sb[:, k : k + 2, D + m * 128 : D + (m + 1) * 128],
                            rhs=xT_sb[:, k : k + 2, ch * WS : (ch + 1) * WS],
                            start=(k == 0),
                            stop=(k == DK - 2),
                            perf_mode=DR,
                        )
                    dst = Kfm[:, m, ch * WS : (ch + 1) * WS]
                    if cfg.use_bk:
                        nc.scalar.activation(
                            out=dst, in_=ps, func=AF.Identity,
                            bias=bk_sb[:, m : m + 1], scale=1.0,
                        )
                    elif (on_act if not split else (ch < 2)):
                        nc.scalar.copy(dst, ps)
                    else:
                        nc.vector.tensor_copy(dst, ps)

            def emit_q(m, chs=None):
                for ch in (range(NQ // WQ) if chs is None else chs):
                    ps = pps.tile([128, WQ], F32, tag="ps")
                    for k in range(0, DK, 2):
                        nc.tensor.matmul(
                            out=ps,
                            lhsT=wqkv_sb[:, k : k + 2, m * 128 : (m + 1) * 128],
                            rhs=xqT_sb[:, k : k + 2, ch * WQ : (ch + 1) * WQ],
                            start=(k == 0),
                            stop=(k == DK - 2),
                            perf_mode=DR,
                        )
                    dst = Qfm[:, m, ch * WQ : (ch + 1) * WQ]
                    if cfg.use_bq:
                        nc.scalar.activation(
                            out=dst, in_=ps, func=AF.Identity,
                            bias=bq_sb[:, m : m + 1], scale=1.0,
                        )
                    else:
                        nc.vector.tensor_copy(dst, ps)

            def emit_v(t0, t1, on_act=False):
                for t in range(t0, t1):
                    ps = pps.tile([128, D], F32, tag="ps")
                    for k in range(0, DK, 2):
                        nc.tensor.matmul(
                            out=ps,
                            lhsT=xT_sb[:, k : k + 2, t * 128 : (t + 1) * 128],
                            rhs=wqkv_sb[:, k : k + 2, 2 * D : 3 * D],
                            start=(k == 0),
                            stop=(k == DK - 2),
                            perf_mode=DR,
                        )
                    vdst = Vaug[:, t, :].rearrange("p (h c) -> p h c", h=H)
                    nc.gpsimd.memset(vdst[:, :, HD:VW], DENOM_COL)
                    if on_act:
                        nc.scalar.activation(
                            out=vdst[:, :, 0:HD],
                            in_=ps.rearrange("p (h c) -> p h c", h=H),
                            func=AF.Identity, scale=VSCALE / WSCALE,
                        )
                    else:
                        nc.vector.tensor_scalar_mul(
                            vdst[:, :, 0:HD],
                            ps.rearrange("p (h c) -> p h c", h=H),
                            VSCALE / WSCALE,
                        )

            def emit_outproj(s, pool=None, work=None, tag="ps"):
                # out-proj + residual + LN1 stats for slot s's two token tiles
                for t in range(2 * s, 2 * s + 2):
                    ps = (pool or pps).tile([128, D], F32, name="ps", tag=tag)
                    for m in range(0, DK, 2):
                        nc.tensor.matmul(
                            out=ps,
                            lhsT=ctx_fm[:, m : m + 2, t * 128 : (t + 1) * 128],
                            rhs=wo_sb[:, m : m + 2, :],
                            start=(m == 0),
                            stop=(m == DK - 2),
                            perf_mode=DR,
                        )
                    nc.vector.scalar_tensor_tensor(
                        out=tt_all[:, t, :], in0=ps,
                        scalar=1.0 / (CTX_SCALE * WSCALE),
                        in1=xown_sb[:, t, :], op0=ALU.mult, op1=ALU.add,
                    )
                    stats = (work or pbw_holder[0]).tile(
                        [128, nc.vector.BN_STATS_DIM], F32, tag="st")
                    nc.vector.bn_stats(out=stats, in_=tt_all[:, t, :])
                    nc.vector.bn_aggr(out=mv_all[:, t, :], in_=stats)

            def emit_ln1_rstd(t0, t1, work):
                # batched rstd for token tiles [t0, t1): one ACT sqrt + DVE recip
                sd = work.tile([128, t1 - t0], F32, name="sd", tag="sd")
                nc.scalar.activation(out=sd, in_=mv_all[:, t0:t1, 1],
                                     func=AF.Sqrt, bias=eps_sb)
                nc.vector.reciprocal(out=rstd_all[:, t0:t1], in_=sd)

            def emit_ln1_rstd_dve(t0, t1, work):
                # DVE-only rstd (fast-inverse-sqrt seed + 2 Newton steps):
                # avoids switching the ACT table away from exp mid-stream
                n = t1 - t0
                ve = work.tile([128, n], F32, name="ve", tag="ve")
                nc.vector.tensor_scalar(out=ve, in0=mv_all[:, t0:t1, 1],
                                        scalar1=EPS, scalar2=None, op0=ALU.add)
                tn = work.tile([128, n], F32, name="tn", tag="tn")
                yn = work.tile([128, n], F32, name="yn", tag="yn")
                wn = work.tile([128, n], F32, name="wn", tag="wn")
                nc.vector.tensor_scalar(out=tn.bitcast(I32), in0=ve.bitcast(I32),
                                        scalar1=1, scalar2=None,
                                        op0=ALU.logical_shift_right)
                nc.vector.tensor_scalar(out=yn.bitcast(I32), in0=tn.bitcast(I32),
                                        scalar1=-1, scalar2=0x5F3759DF,
                                        op0=ALU.mult, op1=ALU.add)
                nc.vector.tensor_scalar(out=wn, in0=ve, scalar1=0.5, scalar2=None,
                                        op0=ALU.mult)
                for it in range(2):
                    nc.vector.tensor_mul(tn, yn, yn)
                    nc.vector.tensor_mul(tn, tn, wn)
                    nc.vector.tensor_scalar(out=tn, in0=tn, scalar1=-1.0,
                                            scalar2=1.5, op0=ALU.mult, op1=ALU.add)
                    dst = rstd_all[:, t0:t1] if it == 1 else yn
                    nc.vector.tensor_mul(dst, yn, tn)
                    if it == 0:
                        pass

            def emit_ln1_apply(ts):
                for t in ts:
                    dst = xln1[:, t, :]
                    nc.vector.tensor_scalar(
                        out=dst, in0=tt_all[:, t, :],
                        scalar1=mv_all[:, t, 0:1], scalar2=rstd_all[:, t : t + 1],
                        op0=ALU.subtract, op1=ALU.mult,
                    )
                    if cfg.use_g1:
                        nc.vector.tensor_mul(dst, dst, g1_b)
                    if cfg.use_bn1:
                        nc.vector.tensor_add(dst, dst, bn1_b)

            def emit_ln1_transpose(ts):
                # SBUF->SBUF xbar DMA transpose (no PE/PSUM), then one Pool
                # bf16->fp8 cast for the whole token range
                for t in ts:
                    nc.sync.dma_start_transpose(
                        out=x1tb[:, :, t * 128 : (t + 1) * 128],
                        in_=xln1[:, t, :],
                    )
                c0, c1 = ts[0] * 128, (ts[-1] + 1) * 128
                nc.gpsimd.tensor_copy(x1t[:, :, c0:c1], x1tb[:, :, c0:c1])

            # phase A: K and Q for all m (K m2/m3 + V t0-3 drain on the
            # otherwise-idle ACT), V t4..15 interleaved into phase B below
            emit_k(0, on_act=False); emit_q(0)
            emit_k(1, on_act=False); emit_q(1)
            emit_v(0, 4, on_act=True)

            pa_done = False

            # ---------------- Phase B: attention + fused out-proj/LN1-stats ------
            pbw_holder = [None]
            with (
                tc.tile_pool(name="pb_es", bufs=4) as pbe,
                tc.tile_pool(name="pb_w", bufs=6) as pbw,
            ):
                pbw_holder[0] = pbw
                order = list(reversed(range(NSLOT)))
                for i, s in enumerate(order):
                    Es = E[s]
                    jmask = Es - 4  # first masked k-tile (last 4-group)
                    cps2 = None
                    for h in range(H):
                        if i == 0 and h == 2:
                            # m2/m3 K+Q: slot 3 (extent 4) only reads K chunk
                            # 0 and slot 2 chunk 1; chunks 2/3 defer to the
                            # DVE idle gaps of slots 1/0 which read them
                            emit_k(2, on_act=True, chs=[0, 1]); emit_q(2)
                        elif i == 0 and h == 4:
                            emit_k(3, on_act=True, chs=[0, 1]); emit_q(3)
                        elif i >= 2 and h == 3:
                            emit_k(2, on_act=False, chs=[i])
                            emit_k(3, on_act=False, chs=[i])
                        m = h // HPT
                        off = (h % HPT) * HD
                        if h % 2 == 0:
                            cps2 = pbc.tile([VW, 2, CHUNK], F32, tag="cps")
                        cps = cps2[:, h % 2, :]
                        for j0 in range(0, Es, 4):
                            sc = pbs.tile([128, 4 * CHUNK], F32, tag="sc")
                            for jj in range(4):
                                j = j0 + jj
                                nc.tensor.matmul(
                                    out=sc[:, jj * CHUNK : (jj + 1) * CHUNK],
                                    lhsT=Kfm[off : off + HD, m, j * KT : (j + 1) * KT],
                                    rhs=Qfm[off : off + HD, m, s * CHUNK : (s + 1) * CHUNK],
                                    start=True,
                                    stop=True,
                                )
                            es = pbe.tile([128, 4 * CHUNK], F8, tag="es")
                            nc.scalar.activation(
                                out=es, in_=sc,
                                func=AF.Exp, scale=1.0 / (WSCALE * WSCALE),
                            )
                            if j0 == jmask:
                                nc.gpsimd.tensor_mul(es, es, masks_sb[:, s, :])
                            esj = es.rearrange("p (j q) -> p j q", j=4)
                            for jj in range(0, 4, 2):
                                j = j0 + jj
                                nc.tensor.matmul(
                                    out=cps,
                                    lhsT=Vaug[:, j : j + 2, h * VW : (h + 1) * VW],
                                    rhs=esj[:, jj : jj + 2, :],
                                    start=(j == 0),
                                    stop=(j == Es - 2),
                                    perf_mode=DR,
                                )
                        inv = pbw.tile([1, CHUNK], F32, tag="inv")
                        nc.vector.reciprocal(out=inv, in_=cps[HD : HD + 1, :])
                        invb = pbw.tile([HD, CHUNK], F32, tag="invb")
                        nc.gpsimd.partition_broadcast(invb, inv)
                        cdst = ctx_fm[off : off + HD, m, s * CHUNK : (s + 1) * CHUNK]
                        nc.vector.tensor_mul(cdst, cps[0:HD, :], invb)
                        if cfg.use_bv:
                            nc.scalar.add(cdst, cdst, bv_sb[off : off + HD, m : m + 1])
                    # stage remaining V projections between slots (needed by the
                    # deeper-extent slots processed later)
                    if i == 0:
                        emit_v(4, 8)
                    elif i == 1:
                        emit_v(8, 12)
                    elif i == 2:
                        emit_v(12, S // KT)
                        pa_done = True
                    # out-proj of the PREVIOUS slot (its ctx muls have drained)
                    if i >= 1:
                        emit_outproj(order[i - 1])
                    if i == 2:
                        # shallow slots' tokens: LN1 rstd + apply + transpose
                        # mid-stream (DVE/DMA/Pool only; ACT exp table stays)
                        emit_ln1_rstd_dve(4, QT, pbw)
                        emit_ln1_apply(range(4, QT))
                        emit_ln1_transpose(range(4, QT))
            # slot 0's out-proj happens in the tail, from the pf_h pool, so
            # pps can close as soon as the attention drains

            pa_cm.__exit__(None, None, None)
            pqkv_cm.__exit__(None, None, None)
            pbc_cm.__exit__(None, None, None)
            pbs_cm.__exit__(None, None, None)
            pps_cm.__exit__(None, None, None)
            pfh_cm = tc.tile_pool(name="pf_h", bufs=2, space="PSUM")
            pfh = pfh_cm.__enter__()
            pfy_cm = tc.tile_pool(name="pf_y", bufs=1, space="PSUM")
            pfy = pfy_cm.__enter__()

            # ---------------- Tail: remaining LN1 + transpose + FFN + LN2 --------
            ptw_cm = tc.tile_pool(name="ptw", bufs=3)
            ptw = ptw_cm.__enter__()

            emit_outproj(0, pool=pfh, work=ptw)
            # t4-7 LN1 fully done mid-phase-B; t0-3 now (ACT sqrt preloads the
            # sqrt table for LN2 while the FFN fills the PE)
            emit_ln1_rstd(0, 4, ptw)
            emit_ln1_apply(range(0, 4))
            emit_ln1_transpose(range(0, 4))

            # ---------------- FFN + LN2 + store ----------------------------------
            with (
                tc.tile_pool(name="pf_hb", bufs=2) as pfhb,
                tc.tile_pool(name="pf_w", bufs=6) as pfw,
                tc.tile_pool(name="pf_o", bufs=4) as pfo,
            ):
                hb_tiles = {}
                def emit_ffn1(ch):
                    hb_all = pfhb.tile([128, FK, WQ], F8, name=f"hb{ch}", tag="hb")
                    hb_tiles[ch] = hb_all
                    for f in range(0, FK, 2):
                        hp = pfh.tile([128, 2, WQ], F32, tag="hp")
                        for fi in range(2):
                            for k in range(0, DK, 2):
                                nc.tensor.matmul(
                                    out=hp[:, fi, :],
                                    lhsT=w1_sb[:, k : k + 2, (f + fi) * 128 : (f + fi + 1) * 128],
                                    rhs=x1t[:, k : k + 2, ch * WQ : (ch + 1) * WQ],
                                    start=(k == 0),
                                    stop=(k == DK - 2),
                                    perf_mode=DR,
                                )
                        if cfg.use_b1:
                            nc.scalar.activation(
                                out=hb_all[:, f, :], in_=hp[:, 0, :], func=AF.Relu,
                                bias=b1_sb[:, f : f + 1], scale=1.0,
                            )
                            nc.scalar.activation(
                                out=hb_all[:, f + 1, :], in_=hp[:, 1, :], func=AF.Relu,
                                bias=b1_sb[:, f + 1 : f + 2], scale=1.0,
                            )
                        else:
                            nc.scalar.activation(
                                out=hb_all[:, f : f + 2, :], in_=hp,
                                func=AF.Relu, scale=HB_SCALE / WSCALE,
                            )

                def emit_ffn2(ch, groups=((0, 2), (2, 2))):
                    hb_all = hb_tiles[ch]
                    tt2 = pfw.tile([128, NSUB, D], F32, tag="tt2")
                    mv2 = pfw.tile([128, NSUB, 2], F32, tag="mv2")
                    for g0, gn in groups:
                        yps = [pfy.tile([128, D], F32, name=f"y{i}", tag=f"y{i}")
                               for i in range(gn)]
                        for f in range(0, FK, 2):
                            for s2 in range(gn):
                                sub = g0 + s2
                                c0 = sub * 128
                                nc.tensor.matmul(
                                    out=yps[s2],
                                    lhsT=hb_all[:, f : f + 2, c0 : c0 + 128],
                                    rhs=w2_sb[:, f : f + 2, :],
                                    start=(f == 0),
                                    stop=(f == FK - 2),
                                    perf_mode=DR,
                                )
                        for s2 in range(gn):
                            sub = g0 + s2
                            nc.vector.scalar_tensor_tensor(
                                out=tt2[:, sub, :], in0=yps[s2],
                                scalar=1.0 / (HB_SCALE * WSCALE),
                                in1=xln1[:, ch * NSUB + sub, :],
                                op0=ALU.mult, op1=ALU.add,
                            )
                            if cfg.use_b2:
                                nc.vector.tensor_add(tt2[:, sub, :], tt2[:, sub, :], b2_b)
                            stats = pfw.tile([128, nc.vector.BN_STATS_DIM], F32, tag="st2")
                            nc.vector.bn_stats(out=stats, in_=tt2[:, sub, :])
                            nc.vector.bn_aggr(out=mv2[:, sub, :], in_=stats)
                        sd2 = pfw.tile([128, 2], F32, name="sd2", tag="sd2")
                        nc.scalar.activation(out=sd2[:, :gn], in_=mv2[:, g0 : g0 + gn, 1],
                                             func=AF.Sqrt, bias=eps_sb)
                        rstd2 = pfw.tile([128, 2], F32, name="rstd2", tag="rstd2")
                        nc.vector.reciprocal(out=rstd2[:, :gn], in_=sd2[:, :gn])
                        for s2 in range(gn):
                            sub = g0 + s2
                            t = ch * NSUB + sub
                            ob = pfo.tile([128, D], F32, tag="ob")
                            nc.vector.tensor_scalar(
                                out=ob, in0=tt2[:, sub, :],
                                scalar1=mv2[:, sub, 0:1], scalar2=rstd2[:, s2 : s2 + 1],
                                op0=ALU.subtract, op1=ALU.mult,
                            )
                            if cfg.use_g2:
                                nc.vector.tensor_mul(ob, ob, g2_b)
                            if cfg.use_bn2:
                                nc.vector.tensor_add(ob, ob, bn2_b)
                            nc.sync.dma_start(
                                out=out_d[t * 128 : (t + 1) * 128, :], in_=ob
                            )

                emit_ffn1(1)
                emit_ffn2(1)
                emit_ffn1(0)
                emit_ffn2(0, groups=((0, 2), (2, 1), (3, 1)))

            pfy_cm.__exit__(None, None, None)
            pfh_cm.__exit__(None, None, None)
            ptw_cm.__exit__(None, None, None)
            pctx_cm.__exit__(None, None, None)

        for _ in range(n_bodies):
            emit_body()

    nc.compile()
    return nc


# ---------------------------------------------------------------------------
# host side
# ---------------------------------------------------------------------------

F8NP = ml_dtypes.float8_e4m3


def build_masks(cfg: Cfg, half: int) -> np.ndarray:
    E = cfg.slot_E()
    qs_l = cfg.slot_qs(half)
    m = np.zeros((128, cfg.NSLOT, 4 * cfg.CHUNK), np.float32)
    k_loc = np.arange(128)[:, None]
    q_loc = np.arange(cfg.CHUNK)[None, :]
    for s, qs in enumerate(qs_l):
        jbase = E[s] - 4
        for jj in range(4):
            j = jbase + jj
            keep = (qs + q_loc) >= (j * cfg.KT + k_loc)
            m[:, s, jj * cfg.CHUNK : (jj + 1) * cfg.CHUNK] = keep
    return m.astype(F8NP)


def host_prepare(inputs: dict, cfg: Cfg):
    """Returns (in_maps, own_idx_per_core)."""
    x = np.asarray(inputs["x"], np.float32)
    Wqkv = np.asarray(inputs["Wqkv"], np.float32)
    bqkv = np.asarray(inputs["bqkv"], np.float32)
    Wo = np.asarray(inputs["Wo"], np.float32)
    bo = np.asarray(inputs["bo"], np.float32)
    W1 = np.asarray(inputs["W1"], np.float32)
    b1 = np.asarray(inputs["b1"], np.float32)
    W2 = np.asarray(inputs["W2"], np.float32)
    b2 = np.asarray(inputs["b2"], np.float32)
    g1 = np.asarray(inputs["g1"], np.float32)
    bn1 = np.asarray(inputs["bn1"], np.float32)
    g2 = np.asarray(inputs["g2"], np.float32)
    bn2 = np.asarray(inputs["bn2"], np.float32)

    D = cfg.D
    scale = 1.0 / np.sqrt(np.float32(cfg.HD))
    wqkvT = np.concatenate(
        [
            np.ascontiguousarray(Wqkv[0:D].T) * scale,
            np.ascontiguousarray(Wqkv[D : 2 * D].T),
            np.ascontiguousarray(Wqkv[2 * D : 3 * D].T),
        ],
        axis=1,
    ).astype(np.float32) * WSCALE
    woT = np.ascontiguousarray(Wo.T) * WSCALE
    w1T = np.ascontiguousarray(W1.T) * WSCALE
    w2T = np.ascontiguousarray(W2.T) * WSCALE
    bq = bqkv[0:D] * scale
    bk = bqkv[D : 2 * D]
    bv = bqkv[2 * D : 3 * D]

    masks = [build_masks(cfg, half) for half in (0, 1)]

    in_maps = []
    own_idx_per_core = []
    for c in range(2 * cfg.B):
        b = c // 2
        half = c % 2
        own_idx = np.concatenate(
            [np.arange(qs, qs + cfg.CHUNK) for qs in cfg.slot_qs(half)]
        )
        own_idx_per_core.append(own_idx)
        xb = x[b]
        in_maps.append(
            {
                "xT": np.ascontiguousarray(xb.T).astype(F8NP),
                "xqT": np.ascontiguousarray(xb[own_idx].T).astype(F8NP),
                "xown": np.ascontiguousarray(xb[own_idx]) + bo[None, :],
                "wqkvT": wqkvT.astype(F8NP),
                "woT": woT.astype(F8NP),
                "w1T": w1T.astype(F8NP),
                "w2T": w2T.astype(F8NP),
                "bq": bq,
                "bk": bk,
                "bv": bv,
                "b1": b1,
                "b2": np.tile(b2[None, :], (128, 1)),
                "g1v": np.tile(g1[None, :], (128, 1)),
                "bn1v": np.tile(bn1[None, :], (128, 1)),
                "g2v": np.tile(g2[None, :], (128, 1)),
                "bn2v": np.tile(bn2[None, :], (128, 1)),
                "masks": masks[half],
            }
        )
    return in_maps, own_idx_per_core


def make_cfg(inputs: dict) -> Cfg:
    x = np.asarray(inputs["x"])
    B, S, D = x.shape
    F = np.asarray(inputs["W1"]).shape[0]
    bqkv = np.asarray(inputs["bqkv"], np.float32)
    cfg = Cfg(
        B=B, S=S, D=D, F=F,
        use_bq=bool(np.any(bqkv[0:D])),
        use_bk=bool(np.any(bqkv[D : 2 * D])),
        use_bv=bool(np.any(bqkv[2 * D : 3 * D])),
        use_b1=bool(np.any(np.asarray(inputs["b1"]))),
        use_b2=bool(np.any(np.asarray(inputs["b2"]))),
        use_g1=not bool(np.all(np.asarray(inputs["g1"]) == 1.0)),
        use_bn1=bool(np.any(np.asarray(inputs["bn1"]))),
        use_g2=not bool(np.all(np.asarray(inputs["g2"]) == 1.0)),
        use_bn2=bool(np.any(np.asarray(inputs["bn2"]))),
    )
    return cfg


_NC_CACHE: dict = {}

TRACE = False
LAST_RESULT = None


def kernel(**inputs) -> np.ndarray:
    global LAST_RESULT
    cfg = make_cfg(inputs)
    key = tuple(sorted(cfg.__dict__.items()))
    if key not in _NC_CACHE:
        _NC_CACHE[key] = build_nc(cfg)
    nc = _NC_CACHE[key]

    in_maps, own_idx_per_core = host_prepare(inputs, cfg)
    ncores = 2 * cfg.B
    res = run_bass_kernel_spmd(
        nc, in_maps, core_ids=list(range(ncores)), trace=TRACE
    )
    LAST_RESULT = res

    out = np.empty((cfg.B, cfg.S, cfg.D), np.float32)
    for c in range(ncores):
        out[c // 2, own_idx_per_core[c]] = res.results[c]["out"]
    return out
